# revision 45
# baseline (speedup 1.0000x reference)
"""NodeEquilibriumLoss Trainium2 kernel.

residual[b] = (EA[b] * e[b]) @ S - q[b] - r[b];  out = mean(residual^2)

S[elem, 2*node+c] = sum_k [elem_ids[k]==elem][node_ids[k]==node] * vecs[k, c]
is the fixed sparse linear map implementing the reference's gather+scatter-add.

Sharding: data-parallel over batch, 8 cores x 512 rows. Per core:
  - S is held in fp8e4m3 (accuracy: ~1.7e-3 rel err on the final MSE, well
    inside tolerance) and built ON DEVICE by gpsimd.local_scatter in
    uint16-packed form: each node's two vector components live at adjacent
    columns (2n, 2n+1), i.e. one little-endian uint16 cell, which halves the
    scatter area vs a bf16 S and quarters it vs scattering components
    separately (~24us of Pool time, fully hidden under the load).
  - the matmul runs in fp8 DoubleRow perf mode: each instruction contracts a
    256-row k-pair (two 128-partition tiles addressed via a [2] free dim on
    both operands), twice the bf16 MAC rate.
  - per 128-row batch tile: axial = EA*e (DVE, bf16 out), transposed on the
    PE against a DMA'd identity (4 transposes assemble each PSUM bank; the
    Activation engine downcasts banks to fp8 axT [128,16,128]), then 8
    DoubleRow matmuls per output block accumulate K=2048 in PSUM.
  - drain: d = psum - r - q on DVE (r first: it loads first), per-partition
    sum(d^2) via Activation Square with accum_out (DVE stt for the final
    block so the last two squares run on parallel engines).
  - output: [128, 16] per-partition partial sums; host reduces in fp64.

Schedule notes (cost-model driven): DMA is the bottleneck -- 16MB of
mandatory f32 input traffic/core = ~47us at 360B/ns -- so the makespan is
the input stream plus a short tail. One eager, data-independent DMA queue
(EA0/e0 first, q/r last, h2-major, the final q split per block) keeps the
8-slot HWDGE completion-semaphore ring fluid; transposes live on the PE
instead of the DMA xbar to keep them out of both the DMA budget and the
semaphore ring.
"""

import numpy as np
import ml_dtypes

B, NE, NN, E2 = 4096, 2048, 1024, 4096
N2 = 2 * NN
NCORES = 8
SHARD = B // NCORES   # 512
BT = 128              # batch rows per tile
NT = SHARD // BT      # 4 batch tiles per core
KT = NE // 128        # 16 contraction tiles of 128
KP = KT // 2          # 8 DoubleRow k-pairs of 256
H2 = 2                # column chunks of 1024 fp8 cols (512 uint16 cells)
W2 = 2                # 512-col output blocks per chunk
NFREE = 512           # output cols per PSUM group
CELLS = 512           # uint16 cells per chunk per i-row
NIDX = 16             # padded scatter entries per (partition, kp, h2) bucket
                      # (max occupancy is 8; 16 keeps each partition's table
                      # row at 512B, the no-penalty DMA descriptor size)

_CACHE = {}


def _build_bass():
    from concourse import bacc
    import concourse.mybir as mybir
    import concourse.tile as tile

    f32 = mybir.dt.float32
    bf16 = mybir.dt.bfloat16
    fp8 = mybir.dt.float8e4
    u16 = mybir.dt.uint16
    i16 = mybir.dt.int16
    mult = mybir.AluOpType.mult
    sub = mybir.AluOpType.subtract
    add = mybir.AluOpType.add
    Square = mybir.ActivationFunctionType.Square
    Copy = mybir.ActivationFunctionType.Copy
    DR = mybir.MatmulPerfMode.DoubleRow

    nc = bacc.Bacc("TRN2", target_bir_lowering=False, debug=False,
                   num_devices=NCORES)
    EA = nc.dram_tensor("EA", [SHARD, NE], bf16, kind="ExternalInput").ap()
    ee = nc.dram_tensor("e", [SHARD, NE], bf16, kind="ExternalInput").ap()
    qq = nc.dram_tensor("q", [SHARD, N2], bf16, kind="ExternalInput").ap()
    rr = nc.dram_tensor("r", [SHARD, N2], bf16, kind="ExternalInput").ap()
    sidx = nc.dram_tensor("sidx", [128, KP, H2, NIDX], i16,
                          kind="ExternalInput").ap()
    sval = nc.dram_tensor("sval", [128, KP, H2, NIDX], u16,
                          kind="ExternalInput").ap()
    sdense = nc.dram_tensor("sdense", [128, 2, 2, 2 * CELLS], fp8,
                            kind="ExternalInput").ap()
    out = nc.dram_tensor("out", [128, NT * H2 * W2], f32,
                         kind="ExternalOutput").ap()

    with tile.TileContext(nc) as tc:
        with (
            tc.tile_pool(name="sconst", bufs=1) as sconst,
            # io/qr hold every batch tile at once: a WAR wait on a reused
            # buffer would stall the in-order SP DMA queue and starve the
            # DMA engines
            tc.tile_pool(name="io", bufs=4) as io,
            tc.tile_pool(name="qr", bufs=8) as qr,
            tc.tile_pool(name="work", bufs=4) as work,
            tc.tile_pool(name="drain", bufs=3) as drain,
            tc.tile_pool(name="ps", bufs=6, space="PSUM") as psp,
            tc.tile_pool(name="tp", bufs=2, space="PSUM") as tpp,
        ):
            # --- one eager, data-independent DMA stream on the SP queue ---
            # The tile scheduler round-robins 8 HWDGE completion semaphores
            # over DMAs in emission order; a DMA gated on compute mid-stream
            # would stall every DMA 8 slots later (measured on the xbar-
            # transpose variant: EA loads waited on transposes).  With only
            # input loads in the ring every wait is satisfied on arrival and
            # the SP sequencer just streams descriptors.  EA0/e0 lead (they
            # feed the longest chain: axial -> transpose -> all of tile 0's
            # matmuls); the small tables ride as one merged DMA behind them.
            id_t = sconst.tile([128, 128], bf16)
            idx_t = sconst.tile([128, KP, H2, NIDX], i16)
            val_t = sconst.tile([128, KP, H2, NIDX], u16)

            # identity for the PE transposes, built on DVE (no DMA needed):
            # keep 1.0 where the affine index j - p == 0
            ones_t = work.tile([128, 128], bf16, tag="ones")
            nc.gpsimd.memset(ones_t, 1.0)
            nc.gpsimd.affine_select(
                id_t, ones_t, pattern=[[1, 128]],
                compare_op=mybir.AluOpType.is_equal, fill=0.0,
                base=0, channel_multiplier=-1)

            nc.sync.dma_start(out=idx_t, in_=sidx)
            nc.sync.dma_start(out=val_t, in_=sval)

            ea_ts, e_ts = [], []
            q_ts, r_ts = {}, {}

            def emit_eae(it):
                sl = slice(it * BT, (it + 1) * BT)
                ea_t = io.tile([128, NE], bf16, tag="ea", name=f"ea_{it}")
                e_t = io.tile([128, NE], bf16, tag="e", name=f"e_{it}")
                nc.sync.dma_start(out=ea_t, in_=EA[sl, :])
                nc.sync.dma_start(out=e_t, in_=ee[sl, :])
                ea_ts.append(ea_t)
                e_ts.append(e_t)

            def emit_qr(h2):
                # r loads before q and drains subtract r first: the chain
                # hanging off the very last input DMA is then a single
                # subtract + square
                cs = slice(h2 * 2 * CELLS, (h2 + 1) * 2 * CELLS)
                for it in range(NT):
                    sl = slice(it * BT, (it + 1) * BT)
                    r_t = qr.tile([128, 2 * CELLS], bf16, tag="r",
                                  name=f"r_{it}_{h2}")
                    q_t = qr.tile([128, 2 * CELLS], bf16, tag="q",
                                  name=f"q_{it}_{h2}")
                    if (it, h2) == (NT - 1, H2 - 1):
                        # the very last r and q loads are split per 512-col
                        # block (r halves, then q halves): each drain link
                        # starts at its own half's arrival and the final
                        # chain is one subtract + one square per half, the
                        # squares on parallel engines
                        for src, t in ((rr, r_t), (qq, q_t)):
                            for w in range(W2):
                                ws = slice(h2 * 2 * CELLS + w * NFREE,
                                           h2 * 2 * CELLS + (w + 1) * NFREE)
                                nc.sync.dma_start(
                                    out=t[:, w * NFREE:(w + 1) * NFREE],
                                    in_=src[sl, ws])
                    else:
                        nc.sync.dma_start(out=r_t, in_=rr[sl, cs])
                        nc.sync.dma_start(out=q_t, in_=qq[sl, cs])
                    q_ts[(it, h2)] = q_t
                    r_ts[(it, h2)] = r_t

            # q/r h0 ride between the EA/e pairs: h0 drains then start ~7us
            # earlier, freeing PSUM banks so the h2=1 ladder can track the
            # scatter production instead of bursting after it
            # the last two h2=1 S chunks (kp6, kp7) ride the DMA stream as
            # host-built dense fp8 tiles: the Pool scatter ladder then ends
            # ~3us earlier, and the ladder-gated final stops with it
            dense_tiles = {}
            for it in range(NT):
                emit_eae(it)
            for j, kp in enumerate((KP - 2, KP - 1)):
                st = sconst.tile([128, 2, 2 * CELLS], fp8,
                                 tag=f"Sd_{kp}", name=f"Sd_{kp}")
                nc.sync.dma_start(out=st, in_=sdense[:, j, :, :])
                dense_tiles[kp] = st
            emit_qr(0)
            emit_qr(1)

            # --- S build: 16 local_scatter calls on uint16-packed fp8 ---
            S_tiles = {}
            for h2 in range(H2):
                for kp in range(KP):
                    if h2 == 1 and kp >= KP - 2:
                        S_tiles[(kp, h2)] = dense_tiles[kp]
                        continue
                    st = sconst.tile([128, 2, 2 * CELLS], fp8,
                                     tag=f"S_{kp}_{h2}")
                    nc.gpsimd.local_scatter(
                        out_ap=st[:, :, :].bitcast(u16),
                        data_ap=val_t[:, kp, h2, :],
                        idxs_ap=idx_t[:, kp, h2, :],
                        channels=128, num_elems=2 * CELLS, num_idxs=NIDX,
                    )
                    S_tiles[(kp, h2)] = st

            # --- axial products (DVE) ---
            ax_ts = []
            for it in range(NT):
                ax = work.tile([128, NE], bf16, tag="ax", name=f"ax_{it}")
                nc.vector.tensor_mul(ax, ea_ts[it], e_ts[it])
                ax_ts.append(ax)

            acc = sconst.tile([128, NT * H2 * W2], f32)
            axT2s = [sconst.tile([128, KT, 128], fp8, tag=f"axT2_{it}",
                                 name=f"axT2_{it}")
                     for it in range(NT)]

            def emit_transpose(it):
                # 4 PE transposes assemble one PSUM bank [128, 4x128] bf16;
                # start only on the first (start marks the whole 2KB zero
                # region), the rest accumulate onto pending-zero bytes.
                # The Activation engine then downcasts the bank to fp8.
                for g in range(KT // 4):
                    tp = tpp.tile([128, 4, 128], bf16, tag="tp",
                                  name=f"tp_{it}_{g}")
                    for j in range(4):
                        kt = 4 * g + j
                        nc.tensor.matmul(
                            tp[:, j, :],
                            lhsT=ax_ts[it][:, kt * 128:(kt + 1) * 128],
                            rhs=id_t[:, :],
                            start=(j == 0), stop=(j == 3),
                            is_transpose=True, skip_group_check=True,
                        )
                    nc.scalar.activation(
                        axT2s[it][:, 4 * g:4 * g + 4, :], tp, Copy)

            def emit_matmuls(it, h2):
                for kp in range(KP):
                    lhsT = axT2s[it][:, 2 * kp:2 * kp + 2, :]
                    for w in range(W2):
                        nb = h2 * W2 + w
                        if kp == 0:
                            ps_tiles[(it, nb)] = psp.tile(
                                [128, NFREE], f32, tag="ps",
                                name=f"ps_{it}_{nb}")
                        nc.tensor.matmul(
                            ps_tiles[(it, nb)],
                            lhsT=lhsT,
                            rhs=S_tiles[(kp, h2)][:, :,
                                                  w * NFREE:(w + 1) * NFREE],
                            start=(kp == 0),
                            stop=(kp == KP - 1),
                            perf_mode=DR,
                        )

            s_ts = {}

            def emit_sadds(h2):
                # s = q + r on DVE (bf16 2x mode) as the chunks arrive,
                # off the critical path: the post-S-build drain is then a
                # single subtract + square per 512-col block
                for it in range(NT):
                    s_t = drain.tile([128, 2 * CELLS], bf16, tag="s",
                                     name=f"s_{it}_{h2}")
                    nc.vector.tensor_tensor(s_t, q_ts[(it, h2)],
                                            r_ts[(it, h2)], add)
                    s_ts[(it, h2)] = s_t

            def emit_drain(it, h2):
                # d = ps - s per block; squares on ACT, except the late
                # (h2=1) w1 blocks on DVE so the drains of the S-build-gated
                # final stops run on parallel engines
                for w in range(W2):
                    nb = h2 * W2 + w
                    csl = slice(w * NFREE, (w + 1) * NFREE)
                    col = it * H2 * W2 + nb
                    d_t = drain.tile([128, NFREE], f32, tag=f"d{w}",
                                     name=f"d_{it}_{nb}")
                    nc.vector.scalar_tensor_tensor(
                        out=d_t, in0=ps_tiles[(it, nb)], scalar=1.0,
                        in1=s_ts[(it, h2)][:, csl], op0=mult, op1=sub,
                    )
                    if h2 == 1 and w == 1:
                        nc.vector.scalar_tensor_tensor(
                            out=d_t, in0=d_t, scalar=1.0, in1=d_t,
                            op0=mult, op1=mult,
                            accum_out=acc[:, col:col + 1])
                    else:
                        d2_t = drain.tile([128, NFREE], f32, tag="d2",
                                          name=f"d2_{it}_{nb}")
                        nc.scalar.activation(
                            d2_t, d_t, Square,
                            accum_out=acc[:, col:col + 1])

            # PE stream in expected-availability order: each batch tile's
            # transposes (gated on its EA/e arrival + DVE product) directly
            # ahead of its chunk-0 matmuls (gated on the scatter ladder);
            # chunk-1 matmuls last (gated on PSUM frees by chunk-0 drains).
            def emit_matmuls_kpmajor(h2):
                # kp-major across ALL groups: consumption tracks the scatter
                # ladder tile-by-tile, so the last stops trail the final
                # scatter by one 8-matmul burst instead of a serial
                # per-batch-tile replay of the whole ladder
                for kp in range(KP):
                    for it in range(NT):
                        lhsT = axT2s[it][:, 2 * kp:2 * kp + 2, :]
                        for w in range(W2):
                            nb = h2 * W2 + w
                            if kp == 0:
                                ps_tiles[(it, nb)] = psp.tile(
                                    [128, NFREE], f32, tag="ps",
                                    name=f"ps_{it}_{nb}")
                            nc.tensor.matmul(
                                ps_tiles[(it, nb)],
                                lhsT=lhsT,
                                rhs=S_tiles[(kp, h2)][:, :,
                                                      w * NFREE:(w + 1) * NFREE],
                                start=(kp == 0),
                                stop=(kp == KP - 1),
                                perf_mode=DR,
                            )

            ps_tiles = {}
            for it in range(NT):
                emit_transpose(it)
            for it in range(NT):
                emit_matmuls(it, 0)
            emit_sadds(0)
            for it in range(NT):
                emit_drain(it, 0)
            emit_matmuls_kpmajor(1)
            emit_sadds(1)
            for it in range(NT):
                emit_drain(it, 1)

            nc.sync.dma_start(out=out, in_=acc)

    nc.compile()
    return nc


def _get_bass():
    if "nc" not in _CACHE:
        _CACHE["nc"] = _build_bass()
    return _CACHE["nc"]


def _build_tables(vecs, node_ids, elem_ids):
    """uint16-packed fp8 scatter tables for the on-device S build.

    Element row e = kt*128 + p lives on partition p as k-pair kp = kt//2,
    i-row i = kt%2.  Output cols (2n, 2n+1) form uint16 cell n; chunk
    h2 = n//512 with local cell nl = n%512 at flat index i*512 + nl inside
    the [2, 512]-cell scatter region of call (kp, h2).
    """
    fp8 = ml_dtypes.float8_e4m3
    buckets = {}
    for k in range(E2):
        kt, p = divmod(int(elem_ids[k]), 128)
        kp, i = divmod(kt, 2)
        n = int(node_ids[k])
        h2, nl = divmod(n, CELLS)
        key = (p, kp, h2)
        d = buckets.setdefault(key, {})
        cell = i * CELLS + nl
        v = d.get(cell, (0.0, 0.0))
        d[cell] = (v[0] + float(vecs[k, 0]), v[1] + float(vecs[k, 1]))
    sidx = np.full((128, KP, H2, NIDX), -1, dtype=np.int16)
    svpair = np.zeros((128, KP, H2, NIDX, 2), dtype=np.float32)
    for (p, kp, h2), d in buckets.items():
        items = list(d.items())
        assert len(items) <= NIDX, f"bucket overflow: {len(items)} > {NIDX}"
        for j, (cell, (v0, v1)) in enumerate(items):
            sidx[p, kp, h2, j] = cell
            svpair[p, kp, h2, j] = (v0, v1)
    sval = np.ascontiguousarray(svpair.astype(fp8)).view(np.uint16)[..., 0]
    sval = np.ascontiguousarray(sval)
    # dense fp8 tiles for the chunks the kernel loads via DMA instead of
    # building with local_scatter (kp in {KP-2, KP-1}, h2 = 1)
    sdense16 = np.zeros((128, 2, 2 * CELLS), dtype=np.uint16)
    for j, kp in enumerate((KP - 2, KP - 1)):
        for p in range(128):
            for jj in range(NIDX):
                c = sidx[p, kp, 1, jj]
                if c >= 0:
                    sdense16[p, j, c] = sval[p, kp, 1, jj]
    sdense = sdense16.view(ml_dtypes.float8_e4m3).reshape(128, 2, 2, 2 * CELLS)
    return sidx, sval, np.ascontiguousarray(sdense)


def _prep_in_maps(EA, e, q, r, vecs, node_ids, elem_ids):
    bf16 = ml_dtypes.bfloat16
    EA = np.ascontiguousarray(np.asarray(EA).astype(bf16))
    e = np.ascontiguousarray(np.asarray(e).astype(bf16))
    q = np.ascontiguousarray(np.asarray(q).astype(bf16)).reshape(B, N2)
    r = np.ascontiguousarray(np.asarray(r).astype(bf16)).reshape(B, N2)
    vecs = np.asarray(vecs, dtype=np.float32)
    sidx, sval, sdense = _build_tables(vecs, np.asarray(node_ids),
                                       np.asarray(elem_ids))

    in_maps = []
    for c in range(NCORES):
        sl = slice(c * SHARD, (c + 1) * SHARD)
        in_maps.append({
            "EA": EA[sl], "e": e[sl], "q": q[sl], "r": r[sl],
            "sidx": sidx, "sval": sval, "sdense": sdense,
        })
    return in_maps


def _reduce_outs(results):
    total = 0.0
    for c in range(NCORES):
        total += results[c]["out"].astype(np.float64).sum()
    return np.array(total / (B * NN * 2), dtype=np.float32)


def kernel_run(EA, e, q, r, vecs, node_ids, elem_ids, trace=False):
    from concourse.bass_utils import run_bass_kernel_spmd

    nc = _get_bass()
    in_maps = _prep_in_maps(EA, e, q, r, vecs, node_ids, elem_ids)
    res = run_bass_kernel_spmd(nc, in_maps, core_ids=list(range(NCORES)),
                               trace=trace)
    return _reduce_outs(res.results), res


def kernel(EA, e, q, r, vecs, node_ids, elem_ids):
    val, _ = kernel_run(EA, e, q, r, vecs, node_ids, elem_ids, trace=False)
    return val
